# revision 5
# baseline (speedup 1.0000x reference)
"""AdaptiveCoverageAttention TRN2 kernel: 8-way (batch x head-group) sharded.

Sharding: core c in 0..7 -> batch b = c//4, head-group hg = c%4 (4 heads each).
Each core computes its 4 heads' attention + its partial output projection;
the host sums the 4 partials per batch (and adds b_out). No collectives.

v2: PE array tiling. S^T uses row-tiled concurrent head pairs (K=64 at
tile_position (0,0)/(64,0)), AV uses col-tiled M=64 pairs ((0,0)/(0,64)),
softmax denominators via two concurrent K=128/M=1 col tiles into spare
PSUM partitions {0,32}. Attention steady state is ScalarE(exp)-bound;
PE work per j-tile is ~1.3us vs the 2.2us exp budget. Pooled-mean
reductions run on GpSimd; normalization uses reciprocal_approx_fast.
"""
import sys

sys.path.insert(0, "/opt/trn_rl_repo")

import numpy as np

B, N, D, H = 2, 2048, 1024, 16
HD = D // H            # 64
HPC = 4                # heads per core
NCORES = 8
IC_W = 1024            # i-chunk width in the attention inner loop

_COMPILED = {}


def _bf16(x):
    import ml_dtypes
    return np.ascontiguousarray(np.asarray(x, np.float32)).astype(ml_dtypes.bfloat16)


def build(n=N):
    """Build the per-core Bass graph for sequence length n (n % 1024 == 0)."""
    import contextlib

    import concourse.bacc as bacc
    import concourse.tile as tile
    from concourse import mybir

    f32 = mybir.dt.float32
    bf16 = mybir.dt.bfloat16
    AFT = mybir.ActivationFunctionType

    NJ = n // 128          # 16 j-tiles (key positions)
    NI = n // 512          # 4 512-wide chunks (projection phase)
    NI2 = n // IC_W        # 2 i-chunks (query positions, attention phase)
    DC = D // 128          # 8 contraction chunks

    nc = bacc.Bacc("TRN2", target_bir_lowering=False, debug=False,
                   num_devices=NCORES)

    dram = lambda name, shape, dt, kind: nc.dram_tensor(name, shape, dt, kind=kind).ap()
    XT = dram("xT", [D, n], bf16, "ExternalInput")
    WQK = dram("wqk", [D, 512], bf16, "ExternalInput")
    WV = dram("wv", [D, 256], bf16, "ExternalInput")
    WO = dram("wo", [256, D], bf16, "ExternalInput")
    COVT = dram("covT", [1, n], bf16, "ExternalInput")
    WCE1 = dram("wce1", [1, 256], bf16, "ExternalInput")
    BCE1 = dram("bce1", [128, 2], f32, "ExternalInput")
    WCE2 = dram("wce2", [128, 8], bf16, "ExternalInput")
    BCE2 = dram("bce2", [128, 4], f32, "ExternalInput")
    WFG1 = dram("wfg1", [D, 256], f32, "ExternalInput")
    BFG1 = dram("bfg1", [128, 2], f32, "ExternalInput")
    WFG2 = dram("wfg2", [128, 2], f32, "ExternalInput")
    BFG2 = dram("bfg2", [1, 1], f32, "ExternalInput")
    OUT = dram("out", [n, D], bf16, "ExternalOutput")

    with tile.TileContext(nc) as tc, contextlib.ExitStack() as ctx:
        consts = ctx.enter_context(tc.tile_pool(name="consts", bufs=1))
        xtp = ctx.enter_context(tc.tile_pool(name="xtp", bufs=DC))
        qkv = ctx.enter_context(tc.tile_pool(name="qkv", bufs=1))
        big2 = ctx.enter_context(tc.tile_pool(name="big2", bufs=1))
        ep = ctx.enter_context(tc.tile_pool(name="ep", bufs=6))
        rp = ctx.enter_context(tc.tile_pool(name="rp", bufs=8))
        wfp = ctx.enter_context(tc.tile_pool(name="wfp", bufs=8))
        yp = ctx.enter_context(tc.tile_pool(name="yp", bufs=4))

        # ---- constants into SBUF ----
        wqk_sb = consts.tile([128, DC, 512], bf16)
        wv_sb = consts.tile([128, DC, 256], bf16)
        wo_sb = consts.tile([128, 2, D], bf16)
        covT_sb = consts.tile([1, n], bf16)
        wce1_sb = consts.tile([1, 256], bf16)
        bce1_sb = consts.tile([128, 2], f32)
        wce2_sb = consts.tile([128, 8], bf16)
        bce2_sb = consts.tile([128, 4], f32)
        bfg1_sb = consts.tile([128, 2], f32)
        wfg2_sb = consts.tile([128, 2], f32)
        bfg2_sb = consts.tile([1, 1], f32)
        for dc in range(DC):
            nc.sync.dma_start(out=wqk_sb[:, dc, :], in_=WQK[dc * 128:(dc + 1) * 128, :])
            nc.sync.dma_start(out=wv_sb[:, dc, :], in_=WV[dc * 128:(dc + 1) * 128, :])
        for pt in range(2):
            nc.sync.dma_start(out=wo_sb[:, pt, :], in_=WO[pt * 128:(pt + 1) * 128, :])
        nc.sync.dma_start(out=covT_sb, in_=COVT)
        nc.sync.dma_start(out=wce1_sb, in_=WCE1)
        nc.sync.dma_start(out=bce1_sb, in_=BCE1)
        nc.sync.dma_start(out=wce2_sb, in_=WCE2)
        nc.sync.dma_start(out=bce2_sb, in_=BCE2)
        nc.sync.dma_start(out=bfg1_sb, in_=BFG1)
        nc.sync.dma_start(out=wfg2_sb, in_=WFG2)
        nc.sync.dma_start(out=bfg2_sb, in_=BFG2)

        ones_f = consts.tile([1, 128], f32)
        nc.vector.memset(ones_f, 1.0)
        ones_bf = consts.tile([128, 1], bf16)
        nc.vector.memset(ones_bf, 1.0)

        pooled_sb = consts.tile([128, DC], f32)
        hidg_sb = consts.tile([128, 2], f32)
        g_sb = consts.tile([1, 1], f32)
        gb_sb = consts.tile([128, 1], f32)
        bias_sb = consts.tile([128, NJ, 4], f32)

        # ---- stage A: load xT, pooled sums, gate/cov MLPs, Q^T/K^T, V ----
        xts = []
        for dc in range(DC):
            xt = xtp.tile([128, n], bf16, tag="xt", name=f"xt{dc}")
            nc.sync.dma_start(out=xt, in_=XT[dc * 128:(dc + 1) * 128, :])
            xts.append(xt)

        with tc.tile_pool(name="psA", bufs=3, space="PSUM") as psA, \
             tc.tile_pool(name="pst", bufs=1, space="PSUM") as pst:
            for dc in range(DC):
                nc.vector.reduce_sum(pooled_sb[:, dc:dc + 1], xts[dc],
                                     axis=mybir.AxisListType.X)

            # gate MLP (tiny, plain f32 matmuls); wfg1 streamed per d-chunk
            wfs = []
            for dc in range(DC):
                wf = wfp.tile([128, 256], f32, tag="wfg1", name=f"wf{dc}")
                nc.sync.dma_start(out=wf, in_=WFG1[dc * 128:(dc + 1) * 128, :])
                wfs.append(wf)
            pg = pst.tile([128, 512], f32, tag="tiny", name="pg")
            for mc in range(2):
                for dc in range(DC):
                    nc.tensor.matmul(pg[:, mc:mc + 1],
                                     wfs[dc][:, mc * 128:(mc + 1) * 128],
                                     pooled_sb[:, dc:dc + 1],
                                     start=(dc == 0), stop=(dc == DC - 1))
            for mc in range(2):
                nc.scalar.activation(out=hidg_sb[:, mc:mc + 1], in_=pg[:, mc:mc + 1],
                                     func=AFT.Silu, bias=bfg1_sb[:, mc:mc + 1],
                                     scale=1.0 / n)
            pgp = pst.tile([128, 512], f32, tag="tiny")
            for mc in range(2):
                nc.tensor.matmul(pgp[0:1, 0:1], hidg_sb[:, mc:mc + 1],
                                 wfg2_sb[:, mc:mc + 1],
                                 start=(mc == 0), stop=(mc == 1))
            nc.scalar.activation(out=g_sb, in_=pgp[0:1, 0:1], func=AFT.Sigmoid,
                                 bias=bfg2_sb, scale=1.0)
            pgb = pst.tile([128, 512], f32, tag="tiny")
            nc.tensor.matmul(pgb[:, 0:1], ones_f, g_sb, start=True, stop=True)
            nc.vector.tensor_copy(gb_sb, pgb[:, 0:1])

            # coverage MLP (tiny, plain f32): hidden^T then cov (scaled by g)
            hidc_sb = big2.tile([128, 2, n], bf16, tag="big", name="hidc")
            for mc in range(2):
                for jc in range(NI):
                    ph = pst.tile([128, 512], f32, tag="tiny")
                    nc.tensor.matmul(ph, wce1_sb[:, mc * 128:(mc + 1) * 128],
                                     covT_sb[:, jc * 512:(jc + 1) * 512],
                                     start=True, stop=True)
                    nc.scalar.activation(out=hidc_sb[:, mc, jc * 512:(jc + 1) * 512],
                                         in_=ph, func=AFT.Silu,
                                         bias=bce1_sb[:, mc:mc + 1], scale=1.0)
            for jt in range(NJ):
                pc = pst.tile([128, 512], f32, tag="tiny")
                for mc in range(2):
                    nc.tensor.matmul(pc[:, 0:4], hidc_sb[:, mc, jt * 128:(jt + 1) * 128],
                                     wce2_sb[:, mc * 4:(mc + 1) * 4],
                                     start=(mc == 0), stop=(mc == 1))
                nc.vector.tensor_add(bias_sb[:, jt, :], pc[:, 0:4], bce2_sb)
            for jt in range(NJ):
                nc.vector.tensor_scalar_mul(out=bias_sb[:, jt, :],
                                            in0=bias_sb[:, jt, :], scalar1=gb_sb)

            # V (natural layout): one cast per j-tile, heads contiguous
            vsb = qkv.tile([128, NJ, 256], bf16)
            for it in range(NJ):
                pv = psA.tile([128, 256], f32, tag="v", name=f"pv{it}", bufs=1)
                for dc in range(DC):
                    nc.tensor.matmul(pv, xts[dc][:, it * 128:(it + 1) * 128],
                                     wv_sb[:, dc, :],
                                     start=(dc == 0), stop=(dc == DC - 1))
                nc.vector.tensor_copy(vsb[:, it, :], pv)

            # Q^T / K^T: [col, i] = sum_d wqk[d, col] * xT[d, i]
            # cb 2,3 are the K pair blocks (computed first), 0,1 the Q pairs.
            # Partition layout per pair: rows 0-63 = head 2p, 64-127 = head 2p+1.
            qt_sb = qkv.tile([128, 2, n], bf16)
            ktp_sb = qkv.tile([128, 2, n], bf16)
            PAIR = 2 if NI % 2 == 0 else 1
            for cb in (2, 3, 0, 1):
                for icp in range(NI // PAIR):
                    pqs = [psA.tile([128, 512], f32, tag="qk",
                                    name=f"pq{cb}_{icp}_{i}", bufs=2)
                           for i in range(PAIR)]
                    for dc in range(DC):
                        for i2 in range(PAIR):
                            ic = icp * PAIR + i2
                            nc.tensor.matmul(pqs[i2],
                                             wqk_sb[:, dc, cb * 128:(cb + 1) * 128],
                                             xts[dc][:, ic * 512:(ic + 1) * 512],
                                             start=(dc == 0), stop=(dc == DC - 1))
                    for i2 in range(PAIR):
                        ic = icp * PAIR + i2
                        dst = (ktp_sb[:, cb - 2, ic * 512:(ic + 1) * 512] if cb >= 2
                               else qt_sb[:, cb, ic * 512:(ic + 1) * 512])
                        nc.vector.tensor_copy(dst, pqs[i2])

        # ---- attention: row-tiled S pairs, exp, col-tiled AV pairs + denoms ----
        scale = float(HD) ** -0.5
        attn_sb = big2.tile([128, 2, n], bf16, tag="big", name="attn")
        with tc.tile_pool(name="pso", bufs=2, space="PSUM") as pso, \
             tc.tile_pool(name="pss", bufs=2, space="PSUM") as pss:
            for p in range(2):
                for ic in range(NI2):
                    po = pso.tile([128, IC_W], f32, tag="o", name=f"po{p}_{ic}",
                                  bufs=1)
                    pd = pso.tile([128, IC_W], f32, tag="d", name=f"pd{p}_{ic}",
                                  bufs=1)
                    for jt in range(NJ):
                        js = slice(jt * 128, (jt + 1) * 128)
                        psa = pss.tile([128, IC_W], f32, tag="s",
                                       name=f"sA{p}_{ic}_{jt}")
                        psb = pss.tile([128, IC_W], f32, tag="s",
                                       name=f"sB{p}_{ic}_{jt}")
                        for q in range(IC_W // 512):
                            qs = slice(q * 512, (q + 1) * 512)
                            ms = slice(ic * IC_W + q * 512, ic * IC_W + (q + 1) * 512)
                            nc.tensor.matmul(psa[:, qs], ktp_sb[0:64, p, js],
                                             qt_sb[0:64, p, ms],
                                             start=True, stop=True,
                                             tile_position=(0, 0))
                            nc.tensor.matmul(psb[:, qs], ktp_sb[64:128, p, js],
                                             qt_sb[64:128, p, ms],
                                             start=True, stop=True,
                                             tile_position=(64, 0))
                        e0 = ep.tile([128, IC_W], bf16, tag="e",
                                     name=f"e0_{p}_{ic}_{jt}")
                        e1 = ep.tile([128, IC_W], bf16, tag="e",
                                     name=f"e1_{p}_{ic}_{jt}")
                        nc.scalar.activation(out=e0, in_=psa, func=AFT.Exp,
                                             bias=bias_sb[:, jt, 2 * p:2 * p + 1],
                                             scale=scale)
                        nc.scalar.activation(out=e1, in_=psb, func=AFT.Exp,
                                             bias=bias_sb[:, jt, 2 * p + 1:2 * p + 2],
                                             scale=scale)
                        st, sp = (jt == 0), (jt == NJ - 1)
                        h0, h1 = 2 * p, 2 * p + 1
                        for q in range(IC_W // 512):
                            qs = slice(q * 512, (q + 1) * 512)
                            nc.tensor.matmul(po[0:64, qs],
                                             vsb[:, jt, h0 * 64:h0 * 64 + 64],
                                             e0[:, qs], start=st, stop=sp,
                                             tile_position=(0, 0))
                            nc.tensor.matmul(po[64:128, qs],
                                             vsb[:, jt, h1 * 64:h1 * 64 + 64],
                                             e1[:, qs], start=st, stop=sp,
                                             tile_position=(0, 64))
                            nc.tensor.matmul(pd[0:1, qs], ones_bf,
                                             e0[:, qs], start=st, stop=sp,
                                             tile_position=(0, 0))
                            nc.tensor.matmul(pd[32:33, qs], ones_bf,
                                             e1[:, qs], start=st, stop=sp,
                                             tile_position=(0, 32))
                    # normalize: 1/d broadcast over the 64 v-rows of each head
                    d0 = rp.tile([1, IC_W], f32, tag="dc", name=f"d0_{p}_{ic}")
                    d1 = rp.tile([1, IC_W], f32, tag="dc", name=f"d1_{p}_{ic}")
                    nc.vector.tensor_copy(d0, pd[0:1, :])
                    nc.vector.tensor_copy(d1, pd[32:33, :])
                    r0 = rp.tile([1, IC_W], f32, tag="dc", name=f"r0_{p}_{ic}")
                    r1 = rp.tile([1, IC_W], f32, tag="dc", name=f"r1_{p}_{ic}")
                    nc.vector.reciprocal_approx_fast(out=r0, in_=d0)
                    nc.vector.reciprocal_approx_fast(out=r1, in_=d1)
                    rb0 = rp.tile([64, IC_W], f32, tag="rb", name=f"rb0_{p}_{ic}")
                    rb1 = rp.tile([64, IC_W], f32, tag="rb", name=f"rb1_{p}_{ic}")
                    nc.gpsimd.partition_broadcast(rb0, r0)
                    nc.gpsimd.partition_broadcast(rb1, r1)
                    osl = slice(ic * IC_W, (ic + 1) * IC_W)
                    nc.vector.tensor_mul(attn_sb[0:64, p, osl], po[0:64, :], rb0)
                    nc.vector.tensor_mul(attn_sb[64:128, p, osl], po[64:128, :], rb1)

        # ---- output projection: y[i, e] = sum_dim attnT[dim, i] wo[dim, e] ----
        with tc.tile_pool(name="psy", bufs=2, space="PSUM") as psy:
            for it in range(NJ):
                py = psy.tile([128, D], f32, tag="y")
                for pt in range(2):
                    for half in range(2):
                        nc.tensor.matmul(
                            py[:, half * 512:(half + 1) * 512],
                            attn_sb[:, pt, it * 128:(it + 1) * 128],
                            wo_sb[:, pt, half * 512:(half + 1) * 512],
                            start=(pt == 0), stop=(pt == 1))
                y_sb = yp.tile([128, D], bf16, tag="y_sb", name=f"ysb{it}")
                if it % 2 == 0:
                    nc.vector.tensor_copy(y_sb, py)
                else:
                    nc.scalar.copy(y_sb, py)
                nc.sync.dma_start(out=OUT[it * 128:(it + 1) * 128, :], in_=y_sb)

    nc.compile()
    return nc


def make_in_maps(x, coverage, w_qkv, w_out, b_out, w_ce1, b_ce1, w_ce2, b_ce2,
                 w_fg1, b_fg1, w_fg2, b_fg2, n=N):
    f = np.float32
    x = np.asarray(x, f)
    coverage = np.asarray(coverage, f)
    w_qkv = np.asarray(w_qkv, f)
    w_out = np.asarray(w_out, f)
    in_maps = []
    for c in range(NCORES):
        b, hg = divmod(c, 4)
        cs, ce = hg * 256, (hg + 1) * 256
        wq = w_qkv[:, 0 * D + cs:0 * D + ce]
        wk = w_qkv[:, 1 * D + cs:1 * D + ce]
        wv = w_qkv[:, 2 * D + cs:2 * D + ce]
        m = {
            "xT": _bf16(x[b].T),
            "wqk": _bf16(np.concatenate([wq, wk], axis=1)),
            "wv": _bf16(wv),
            "wo": _bf16(w_out[cs:ce, :]),
            "covT": _bf16(coverage[b, :, 0][None, :]),
            "wce1": _bf16(w_ce1),
            "bce1": np.ascontiguousarray(np.asarray(b_ce1, f).reshape(2, 128).T),
            "wce2": _bf16(
                np.asarray(w_ce2, f)[:, 4 * hg:4 * hg + 4].reshape(2, 128, 4)
                .transpose(1, 0, 2).reshape(128, 8)),
            "bce2": np.tile(np.asarray(b_ce2, f)[4 * hg:4 * hg + 4][None, :], (128, 1)),
            "wfg1": np.ascontiguousarray(np.asarray(w_fg1, f)),
            "bfg1": np.ascontiguousarray(np.asarray(b_fg1, f).reshape(2, 128).T),
            "wfg2": np.ascontiguousarray(np.asarray(w_fg2, f).reshape(2, 128).T),
            "bfg2": np.asarray(b_fg2, f).reshape(1, 1),
        }
        in_maps.append(m)
    return in_maps


def kernel(**inputs):
    from concourse.bass_utils import run_bass_kernel_spmd
    if "nc" not in _COMPILED:
        _COMPILED["nc"] = build(N)
    nc = _COMPILED["nc"]
    in_maps = make_in_maps(**inputs)
    res = run_bass_kernel_spmd(nc, in_maps, core_ids=list(range(NCORES)))
    outs = [np.asarray(res.results[c]["out"], dtype=np.float32)
            for c in range(NCORES)]
    b_out = np.asarray(inputs["b_out"], np.float32)
    full = np.stack([
        outs[0] + outs[1] + outs[2] + outs[3] + b_out[None, :],
        outs[4] + outs[5] + outs[6] + outs[7] + b_out[None, :],
    ]).astype(np.float32)
    return full


# revision 9
# speedup vs baseline: 1.1776x; 1.1776x over previous
"""AdaptiveCoverageAttention TRN2 kernel: 8-way (batch x head-group) sharded.

Sharding: core c in 0..7 -> batch b = c//4, head-group hg = c%4 (4 heads each).
Each core computes its 4 heads' attention + its partial output projection;
the host sums the 4 partials per batch (and adds b_out). No collectives.

v2.5: baseline exp-bound attention loop (S double-buffered per head, AV with
a ones column for free softmax denominators), plus: K^T stored as head pairs
(no 64-row splitting, S uses base-partition 0/64 slices), single-cast V with
a memset-1.0 vaug (ones columns for free), reciprocal_approx_fast for the
denominators, chunked xT DMA + K-first projection order for a fast start,
bf16 output with PSUM->SBUF copies split across DVE and ScalarE.
"""
import sys

sys.path.insert(0, "/opt/trn_rl_repo")

import numpy as np

B, N, D, H = 2, 2048, 1024, 16
HD = D // H            # 64
HPC = 4                # heads per core
NCORES = 8
IC_W = 1024

_COMPILED = {}


def _bf16(x):
    import ml_dtypes
    return np.ascontiguousarray(np.asarray(x, np.float32)).astype(ml_dtypes.bfloat16)


def build(n=N):
    """Build the per-core Bass graph for sequence length n (n % 1024 == 0)."""
    import contextlib

    import concourse.bacc as bacc
    import concourse.tile as tile
    from concourse import mybir

    f32 = mybir.dt.float32
    bf16 = mybir.dt.bfloat16
    AFT = mybir.ActivationFunctionType

    NJ = n // 128          # 16 j-tiles (key positions)
    NI = n // 512          # 4 512-wide chunks (projection phase)
    NI2 = n // IC_W        # 2 i-chunks (query positions, attention phase)
    DC = D // 128          # 8 contraction chunks

    nc = bacc.Bacc("TRN2", target_bir_lowering=False, debug=False,
                   num_devices=NCORES)

    dram = lambda name, shape, dt, kind: nc.dram_tensor(name, shape, dt, kind=kind).ap()
    XT = dram("xT", [D, n], bf16, "ExternalInput")
    WQK = dram("wqk", [D, 512], bf16, "ExternalInput")
    WV = dram("wv", [D, 256], bf16, "ExternalInput")
    WO = dram("wo", [256, D], bf16, "ExternalInput")
    COVT = dram("covT", [1, n], bf16, "ExternalInput")
    WCE1 = dram("wce1", [1, 256], bf16, "ExternalInput")
    BCE1 = dram("bce1", [128, 2], f32, "ExternalInput")
    WCE2 = dram("wce2", [128, 8], bf16, "ExternalInput")
    BCE2 = dram("bce2", [128, 4], f32, "ExternalInput")
    WFG1 = dram("wfg1", [D, 256], f32, "ExternalInput")
    BFG1 = dram("bfg1", [128, 2], f32, "ExternalInput")
    WFG2 = dram("wfg2", [128, 2], f32, "ExternalInput")
    BFG2 = dram("bfg2", [1, 1], f32, "ExternalInput")
    OUT = dram("out", [n, D], bf16, "ExternalOutput")

    with tile.TileContext(nc) as tc, contextlib.ExitStack() as ctx:
        consts = ctx.enter_context(tc.tile_pool(name="consts", bufs=1))
        xtp = ctx.enter_context(tc.tile_pool(name="xtp", bufs=DC))
        qkv = ctx.enter_context(tc.tile_pool(name="qkv", bufs=1))
        big2 = ctx.enter_context(tc.tile_pool(name="big2", bufs=1))
        ep = ctx.enter_context(tc.tile_pool(name="ep", bufs=6))
        rp = ctx.enter_context(tc.tile_pool(name="rp", bufs=3))
        wfp = ctx.enter_context(tc.tile_pool(name="wfp", bufs=8))
        yp = ctx.enter_context(tc.tile_pool(name="yp", bufs=4))

        # ---- constants into SBUF ----
        wqk_sb = consts.tile([128, DC, 512], bf16)
        wv_sb = consts.tile([128, DC, 256], bf16)
        wo_sb = consts.tile([128, 2, D], bf16)
        covT_sb = consts.tile([1, n], bf16)
        wce1_sb = consts.tile([1, 256], bf16)
        bce1_sb = consts.tile([128, 2], f32)
        wce2_sb = consts.tile([128, 8], bf16)
        bce2_sb = consts.tile([128, 4], f32)
        bfg1_sb = consts.tile([128, 2], f32)
        wfg2_sb = consts.tile([128, 2], f32)
        bfg2_sb = consts.tile([1, 1], f32)
        for dc in range(DC):
            nc.sync.dma_start(out=wqk_sb[:, dc, :], in_=WQK[dc * 128:(dc + 1) * 128, :])
            nc.sync.dma_start(out=wv_sb[:, dc, :], in_=WV[dc * 128:(dc + 1) * 128, :])
        for pt in range(2):
            nc.sync.dma_start(out=wo_sb[:, pt, :], in_=WO[pt * 128:(pt + 1) * 128, :])
        nc.sync.dma_start(out=covT_sb, in_=COVT)
        nc.sync.dma_start(out=wce1_sb, in_=WCE1)
        nc.sync.dma_start(out=bce1_sb, in_=BCE1)
        nc.sync.dma_start(out=wce2_sb, in_=WCE2)
        nc.sync.dma_start(out=bce2_sb, in_=BCE2)
        nc.sync.dma_start(out=bfg1_sb, in_=BFG1)
        nc.sync.dma_start(out=wfg2_sb, in_=WFG2)
        nc.sync.dma_start(out=bfg2_sb, in_=BFG2)

        ones_f = consts.tile([1, 128], f32)
        nc.vector.memset(ones_f, 1.0)

        pooled_sb = consts.tile([128, DC], f32)
        hidg_sb = consts.tile([128, 2], f32)
        g_sb = consts.tile([1, 1], f32)
        gb_sb = consts.tile([128, 1], f32)
        bias_sb = consts.tile([128, NJ, 4], f32)

        # ---- stage A: load xT (chunked, j-major so K^T can start early) ----
        xts = []
        for dc in range(DC):
            xt = xtp.tile([128, NI, 512], bf16, tag="xt", name=f"xt{dc}")
            xts.append(xt)
        for jc in range(NI):
            for dc in range(DC):
                nc.sync.dma_start(out=xts[dc][:, jc, :],
                                  in_=XT[dc * 128:(dc + 1) * 128,
                                         jc * 512:(jc + 1) * 512])

        with tc.tile_pool(name="psA", bufs=3, space="PSUM") as psA, \
             tc.tile_pool(name="pst", bufs=1, space="PSUM") as pst:
            # Q^T / K^T: [col, i] = sum_d wqk[d, col] * xT[d, i]
            # cb 2,3 are K pair blocks (head pairs in partition halves),
            # 0,1 the matching Q pair blocks.
            qt_sb = qkv.tile([128, 2, n], bf16)
            ktp_sb = qkv.tile([128, 2, n], bf16)
            for cb in (2, 3, 0, 1):
                for icp in range(NI // 2):
                    pqs = [psA.tile([128, 512], f32, tag="qk",
                                    name=f"pq{cb}_{icp}_{i}", bufs=2)
                           for i in range(2)]
                    for dc in range(DC):
                        for i2 in range(2):
                            ic = icp * 2 + i2
                            nc.tensor.matmul(pqs[i2],
                                             wqk_sb[:, dc, cb * 128:(cb + 1) * 128],
                                             xts[dc][:, ic, :],
                                             start=(dc == 0), stop=(dc == DC - 1))
                    for i2 in range(2):
                        ic = icp * 2 + i2
                        dst = (ktp_sb[:, cb - 2, ic * 512:(ic + 1) * 512] if cb >= 2
                               else qt_sb[:, cb, ic * 512:(ic + 1) * 512])
                        nc.vector.tensor_copy(dst, pqs[i2])

            # V (+ ones columns via memset 1.0): vaug [j, 4, 64+1]
            vaug_sb = qkv.tile([128, NJ, 4, 65], bf16)
            nc.vector.memset(vaug_sb, 1.0)
            for it in range(NJ):
                pv = psA.tile([128, 4, 64], f32, tag="v", name=f"pv{it}", bufs=1)
                for dc in range(DC):
                    nc.tensor.matmul(pv, xts[dc][:, it // 4, (it % 4) * 128:
                                                 (it % 4) * 128 + 128],
                                     wv_sb[:, dc, :],
                                     start=(dc == 0), stop=(dc == DC - 1))
                nc.vector.tensor_copy(vaug_sb[:, it, :, 0:64], pv)

            # pooled mean sums for the fusion gate
            for dc in range(DC):
                nc.vector.reduce_sum(pooled_sb[:, dc:dc + 1], xts[dc],
                                     axis=mybir.AxisListType.XY)

            # gate MLP (tiny, plain f32 matmuls); wfg1 streamed per d-chunk
            wfs = []
            for dc in range(DC):
                wf = wfp.tile([128, 256], f32, tag="wfg1", name=f"wf{dc}")
                nc.sync.dma_start(out=wf, in_=WFG1[dc * 128:(dc + 1) * 128, :])
                wfs.append(wf)
            pg = pst.tile([128, 512], f32, tag="tiny", name="pg")
            for mc in range(2):
                for dc in range(DC):
                    nc.tensor.matmul(pg[:, mc:mc + 1],
                                     wfs[dc][:, mc * 128:(mc + 1) * 128],
                                     pooled_sb[:, dc:dc + 1],
                                     start=(dc == 0), stop=(dc == DC - 1))
            for mc in range(2):
                nc.scalar.activation(out=hidg_sb[:, mc:mc + 1], in_=pg[:, mc:mc + 1],
                                     func=AFT.Silu, bias=bfg1_sb[:, mc:mc + 1],
                                     scale=1.0 / n)
            pgp = pst.tile([128, 512], f32, tag="tiny")
            for mc in range(2):
                nc.tensor.matmul(pgp[0:1, 0:1], hidg_sb[:, mc:mc + 1],
                                 wfg2_sb[:, mc:mc + 1],
                                 start=(mc == 0), stop=(mc == 1))
            nc.scalar.activation(out=g_sb, in_=pgp[0:1, 0:1], func=AFT.Sigmoid,
                                 bias=bfg2_sb, scale=1.0)
            pgb = pst.tile([128, 512], f32, tag="tiny")
            nc.tensor.matmul(pgb[:, 0:1], ones_f, g_sb, start=True, stop=True)
            nc.vector.tensor_copy(gb_sb, pgb[:, 0:1])

            # coverage MLP (tiny, plain f32): hidden^T then cov (scaled by g)
            hidc_sb = big2.tile([128, 2, n], bf16, tag="big", name="hidc")
            for mc in range(2):
                for jc in range(NI):
                    ph = pst.tile([128, 512], f32, tag="tiny")
                    nc.tensor.matmul(ph, wce1_sb[:, mc * 128:(mc + 1) * 128],
                                     covT_sb[:, jc * 512:(jc + 1) * 512],
                                     start=True, stop=True)
                    nc.scalar.activation(out=hidc_sb[:, mc, jc * 512:(jc + 1) * 512],
                                         in_=ph, func=AFT.Silu,
                                         bias=bce1_sb[:, mc:mc + 1], scale=1.0)
            for jt in range(NJ):
                pc = pst.tile([128, 512], f32, tag="tiny")
                for mc in range(2):
                    nc.tensor.matmul(pc[:, 0:4], hidc_sb[:, mc, jt * 128:(jt + 1) * 128],
                                     wce2_sb[:, mc * 4:(mc + 1) * 4],
                                     start=(mc == 0), stop=(mc == 1))
                nc.vector.tensor_add(bias_sb[:, jt, :], pc[:, 0:4], bce2_sb)
            for jt in range(NJ):
                nc.vector.tensor_scalar_mul(out=bias_sb[:, jt, :],
                                            in0=bias_sb[:, jt, :], scalar1=gb_sb)

        # ---- attention: per head pair, S^T -> exp -> [V|1]^T P^T ----
        scale = float(HD) ** -0.5
        attn_sb = big2.tile([128, 2, n], bf16, tag="big", name="attn")
        with tc.tile_pool(name="pso", bufs=2, space="PSUM") as pso, \
             tc.tile_pool(name="pss", bufs=2, space="PSUM") as pss, \
             tc.tile_pool(name="unp", bufs=2 * NI2 + 2) as unp:
            for p in range(2):
                for ic in range(NI2):
                    po = [pso.tile([128, IC_W], f32, tag="o",
                                   name=f"po{p}_{ic}_{i}") for i in range(2)]
                    for jt in range(NJ):
                        js = slice(jt * 128, (jt + 1) * 128)
                        pss_t, es = [], []
                        for hh in range(2):
                            lo = hh * 64
                            ps_ = pss.tile([128, IC_W], f32, tag="s",
                                           name=f"s{p}_{ic}_{jt}_{hh}")
                            for q in range(IC_W // 512):
                                nc.tensor.matmul(
                                    ps_[:, q * 512:(q + 1) * 512],
                                    ktp_sb[lo:lo + 64, p, js],
                                    qt_sb[lo:lo + 64, p,
                                          ic * IC_W + q * 512:ic * IC_W + (q + 1) * 512],
                                    start=True, stop=True)
                            pss_t.append(ps_)
                        for hh in range(2):
                            h = 2 * p + hh
                            e = ep.tile([128, IC_W], bf16, tag="e",
                                        name=f"e{p}_{ic}_{jt}_{hh}")
                            nc.scalar.activation(out=e, in_=pss_t[hh], func=AFT.Exp,
                                                 bias=bias_sb[:, jt, h:h + 1],
                                                 scale=scale)
                            es.append(e)
                        st, sp = (jt == 0), (jt == NJ - 1)
                        for hh in range(2):
                            h = 2 * p + hh
                            for q in range(IC_W // 512):
                                nc.tensor.matmul(
                                    po[hh][0:65, q * 512:(q + 1) * 512],
                                    vaug_sb[:, jt, h, :],
                                    es[hh][:, q * 512:(q + 1) * 512],
                                    start=st, stop=sp)
                    # normalize: unn = [O^T; d], 1/d broadcast over 64 v-rows
                    osl = slice(ic * IC_W, (ic + 1) * IC_W)
                    for hh in range(2):
                        lo = hh * 64
                        unn = unp.tile([65, IC_W], f32, tag="unn",
                                       name=f"unn{p}_{ic}_{hh}")
                        nc.vector.tensor_copy(unn, po[hh][0:65, :])
                        dd = rp.tile([1, IC_W], f32, tag="dp",
                                     name=f"dd{p}_{ic}_{hh}")
                        nc.vector.tensor_copy(dd, unn[64:65, :])
                        rr = rp.tile([1, IC_W], f32, tag="rr",
                                     name=f"rr{p}_{ic}_{hh}")
                        nc.vector.reciprocal_approx_fast(out=rr, in_=dd)
                        recb = rp.tile([64, IC_W], f32, tag="recb",
                                       name=f"recb{p}_{ic}_{hh}")
                        nc.gpsimd.partition_broadcast(recb, rr)
                        nc.vector.tensor_mul(attn_sb[lo:lo + 64, p, osl],
                                             unn[0:64, :], recb)

        # ---- output projection: y[i, e] = sum_dim attnT[dim, i] wo[dim, e] ----
        with tc.tile_pool(name="psy", bufs=2, space="PSUM") as psy:
            for it in range(NJ):
                py = psy.tile([128, D], f32, tag="y")
                for pt in range(2):
                    for half in range(2):
                        nc.tensor.matmul(
                            py[:, half * 512:(half + 1) * 512],
                            attn_sb[:, pt, it * 128:(it + 1) * 128],
                            wo_sb[:, pt, half * 512:(half + 1) * 512],
                            start=(pt == 0), stop=(pt == 1))
                y_sb = yp.tile([128, D], bf16, tag="y_sb", name=f"ysb{it}")
                if it % 2 == 0:
                    nc.vector.tensor_copy(y_sb, py)
                else:
                    nc.scalar.copy(y_sb, py)
                nc.sync.dma_start(out=OUT[it * 128:(it + 1) * 128, :], in_=y_sb)

    nc.compile()
    return nc


def make_in_maps(x, coverage, w_qkv, w_out, b_out, w_ce1, b_ce1, w_ce2, b_ce2,
                 w_fg1, b_fg1, w_fg2, b_fg2, n=N):
    f = np.float32
    x = np.asarray(x, f)
    coverage = np.asarray(coverage, f)
    w_qkv = np.asarray(w_qkv, f)
    w_out = np.asarray(w_out, f)
    in_maps = []
    for c in range(NCORES):
        b, hg = divmod(c, 4)
        cs, ce = hg * 256, (hg + 1) * 256
        wq = w_qkv[:, 0 * D + cs:0 * D + ce]
        wk = w_qkv[:, 1 * D + cs:1 * D + ce]
        wv = w_qkv[:, 2 * D + cs:2 * D + ce]
        m = {
            "xT": _bf16(x[b].T),
            "wqk": _bf16(np.concatenate([wq, wk], axis=1)),
            "wv": _bf16(wv),
            "wo": _bf16(w_out[cs:ce, :]),
            "covT": _bf16(coverage[b, :, 0][None, :]),
            "wce1": _bf16(w_ce1),
            "bce1": np.ascontiguousarray(np.asarray(b_ce1, f).reshape(2, 128).T),
            "wce2": _bf16(
                np.asarray(w_ce2, f)[:, 4 * hg:4 * hg + 4].reshape(2, 128, 4)
                .transpose(1, 0, 2).reshape(128, 8)),
            "bce2": np.tile(np.asarray(b_ce2, f)[4 * hg:4 * hg + 4][None, :], (128, 1)),
            "wfg1": np.ascontiguousarray(np.asarray(w_fg1, f)),
            "bfg1": np.ascontiguousarray(np.asarray(b_fg1, f).reshape(2, 128).T),
            "wfg2": np.ascontiguousarray(np.asarray(w_fg2, f).reshape(2, 128).T),
            "bfg2": np.asarray(b_fg2, f).reshape(1, 1),
        }
        in_maps.append(m)
    return in_maps


def kernel(**inputs):
    from concourse.bass_utils import run_bass_kernel_spmd
    if "nc" not in _COMPILED:
        _COMPILED["nc"] = build(N)
    nc = _COMPILED["nc"]
    in_maps = make_in_maps(**inputs)
    res = run_bass_kernel_spmd(nc, in_maps, core_ids=list(range(NCORES)))
    outs = [np.asarray(res.results[c]["out"], dtype=np.float32)
            for c in range(NCORES)]
    b_out = np.asarray(inputs["b_out"], np.float32)
    full = np.stack([
        outs[0] + outs[1] + outs[2] + outs[3] + b_out[None, :],
        outs[4] + outs[5] + outs[6] + outs[7] + b_out[None, :],
    ]).astype(np.float32)
    return full
